# revision 6
# baseline (speedup 1.0000x reference)
"""Trainium2 Bass kernel for AttentiveGraphPooling (gnn_message_passing).

Strategy: shard the 4096 graphs across 8 cores (512 graphs each). batch is
sorted, so each core owns a contiguous node range covering whole graphs ->
pooling / gather / GRU are all core-local, no collectives needed.

Per core, graphs are processed in 4 blocks of 128. Within a block, nodes are
processed in 128-row tiles. A per-tile one-hot matrix E (node x local-graph,
built on the vector engine via iota/is_equal) turns both the segment mean-pool
(E.T @ x, PSUM-accumulated) and the per-node graph-context gather (E @ G) into
TensorEngine matmuls. The gate MLP runs on the TensorEngine with resident
weights; the GRU cell runs per graph-block with biases folded in via K=1
matmuls.
"""

import os
import sys

import numpy as np

sys.path.insert(0, "/opt/trn_rl_repo")

H = 256
NBLK = 4  # graph blocks per core
GBLK = 128  # graphs per block
NUM_TIMESTEPS = 2
CHUNK = 4  # node tiles per DMA


def _build_program(NT, nblk=NBLK, dtype_x="float32"):
    """Build the single-core SPMD Bass program. NT = node tiles per block."""
    from contextlib import ExitStack

    import concourse.bass as bass
    import concourse.tile as tile
    from concourse import bacc, mybir

    fp32 = mybir.dt.float32
    dtx = getattr(mybir.dt, dtype_x)

    NTP = NT * 128  # padded nodes per block

    nc = bacc.Bacc("TRN2", target_bir_lowering=False, debug=False)

    # ---- DRAM parameters (per-core inputs) ----
    x_d = nc.dram_tensor("xk", [nblk * NTP, H], dtx, kind="ExternalInput")
    bcols_d = nc.dram_tensor("bcols", [nblk, 128, NT], fp32, kind="ExternalInput")
    invc_d = nc.dram_tensor("invc", [nblk, GBLK, 1], fp32, kind="ExternalInput")
    w1t_d = nc.dram_tensor("w1t", [2, 128, H], fp32, kind="ExternalInput")
    b1_d = nc.dram_tensor("b1", [2, 128, 1], fp32, kind="ExternalInput")
    w2_d = nc.dram_tensor("w2", [2, 128, 1], fp32, kind="ExternalInput")
    wih_d = nc.dram_tensor("wih_t", [2, 128, 3 * H], fp32, kind="ExternalInput")
    whh_d = nc.dram_tensor("whh_t", [2, 128, 3 * H], fp32, kind="ExternalInput")
    brz_d = nc.dram_tensor("bsum_rz", [1, 2 * H], fp32, kind="ExternalInput")
    bin_d = nc.dram_tensor("bihn", [1, H], fp32, kind="ExternalInput")
    bhn_d = nc.dram_tensor("bhhn", [1, H], fp32, kind="ExternalInput")
    b2_d = nc.dram_tensor("b2", [1, 1], fp32, kind="ExternalInput")
    iota_d = nc.dram_tensor("iota_row", [128, 128], fp32, kind="ExternalInput")
    eye_d = nc.dram_tensor("eye128", [128, 128], fp32, kind="ExternalInput")
    out_d = nc.dram_tensor("out", [nblk * GBLK, H], fp32, kind="ExternalOutput")

    with tile.TileContext(nc) as tc, ExitStack() as ctx:
        ep = ctx.enter_context  # shorthand

        const = ep(tc.tile_pool(name="const", bufs=1))
        xpool = ep(tc.tile_pool(name="xchunk", bufs=3))
        bpool = ep(tc.tile_pool(name="bcols", bufs=2))
        epool = ep(tc.tile_pool(name="eoh", bufs=3))
        etsb = ep(tc.tile_pool(name="etsb", bufs=3))
        gisb = ep(tc.tile_pool(name="gisb", bufs=2))
        h1sb = ep(tc.tile_pool(name="h1sb", bufs=2))
        wxsb = ep(tc.tile_pool(name="wxsb", bufs=2))
        gtsb = ep(tc.tile_pool(name="gtsb", bufs=2))
        gsb = ep(tc.tile_pool(name="gsb", bufs=12))
        smallsb = ep(tc.tile_pool(name="smallsb", bufs=8))

        ps_pool = ep(tc.tile_pool(name="pspool", bufs=1, space="PSUM"))
        ps_et = ep(tc.tile_pool(name="pset", bufs=2, space="PSUM"))
        ps_ctx = ep(tc.tile_pool(name="psctx", bufs=2, space="PSUM"))
        ps_tr = ep(tc.tile_pool(name="pstr", bufs=1, space="PSUM"))
        ps_h1 = ep(tc.tile_pool(name="psh1", bufs=1, space="PSUM"))
        ps_gt = ep(tc.tile_pool(name="psgt", bufs=1, space="PSUM"))

        # ---- load constants ----
        def cload(shape, src, tag, dt=fp32):
            t = const.tile(shape, dt, tag=tag)
            nc.sync.dma_start(t[:], src)
            return t

        iota_row = cload([128, 128], iota_d[:], "c_iota")
        eye = cload([128, 128], eye_d[:], "c_eye")
        w1t = [cload([128, H], w1t_d[k], f"c_w1t{k}") for k in range(2)]
        b1 = [cload([128, 1], b1_d[k], f"c_b1{k}") for k in range(2)]
        w2 = [cload([128, 1], w2_d[k], f"c_w2{k}") for k in range(2)]
        wih = [cload([128, 3 * H], wih_d[k], f"c_wih{k}") for k in range(2)]
        whh = [cload([128, 3 * H], whh_d[k], f"c_whh{k}") for k in range(2)]
        brz = cload([1, 2 * H], brz_d[:], "c_brz")
        bin_ = cload([1, H], bin_d[:], "c_bin")
        bhn = cload([1, H], bhn_d[:], "c_bhn")
        b2t = cload([1, 1], b2_d[:], "c_b2")
        invc = [cload([GBLK, 1], invc_d[j], f"c_invc{j}") for j in range(nblk)]
        ones_row = const.tile([1, 128], fp32)
        nc.vector.memset(ones_row[:], 1.0)
        # broadcast b2 scalar to all 128 partitions (K=1 matmul), keep in SBUF
        b2ps = ps_gt.tile([128, 1], fp32, tag="psgt")
        nc.tensor.matmul(b2ps[:], ones_row[:], b2t[:], start=True, stop=True)
        b2col = const.tile([128, 1], fp32)
        nc.scalar.copy(b2col[:], b2ps[:])

        g_gm = [None] * nblk  # graph_repr, graph-major (g, h)
        g_fm = [None] * nblk  # graph_repr, feature-major (hi, g) x2 chunks

        def load_bcols(j):
            bt = bpool.tile([128, NT], fp32, tag="bcols")
            nc.sync.dma_start(bt[:], bcols_d[j])
            return bt

        def x_stream(j):
            """Yield (tile_idx, x_tile AP) for block j, chunked DMA."""
            nchunk = NT // CHUNK
            for ch in range(nchunk):
                xt = xpool.tile([128, CHUNK, H], dtx, tag="xchunk")
                base = j * NTP + ch * CHUNK * 128
                src = x_d[base : base + CHUNK * 128, :].rearrange(
                    "(c p) h -> p c h", p=128
                )
                nc.sync.dma_start(xt[:], src)
                for c in range(CHUNK):
                    yield ch * CHUNK + c, xt[:, c, :]

        def make_E(bt, t):
            e = epool.tile([128, 128], fp32, tag="eoh")
            nc.vector.tensor_scalar(
                e[:], iota_row[:], bt[:, t : t + 1], None, op0=mybir.AluOpType.is_equal
            )
            return e

        def transpose_to_sbuf(src_ap, pool, tag):
            """PE-transpose a (128, 128) SBUF AP -> SBUF tile via PSUM."""
            tp = ps_et.tile([128, 128], fp32, tag="pset")
            nc.tensor.matmul(tp[:], src_ap, eye[:], is_transpose=True,
                             start=True, stop=True)
            dst = pool.tile([128, 128], fp32, tag=tag)
            nc.scalar.copy(dst[:], tp[:])
            return dst

        # ================= Phase A: initial mean pool =================
        for j in range(nblk):
            bt = load_bcols(j)
            pooled = ps_pool.tile([GBLK, H], fp32, tag="pspool")
            for t, x_ap in x_stream(j):
                e = make_E(bt, t)
                nc.tensor.matmul(
                    pooled[:], e[:], x_ap, start=(t == 0), stop=(t == NT - 1),
                    skip_group_check=True,
                )
            g = gsb.tile([GBLK, H], fp32, tag="gsb")
            nc.vector.tensor_scalar(
                g[:], pooled[:], invc[j][:], None, op0=mybir.AluOpType.mult
            )
            g_gm[j] = g
            # feature-major copy for GRU lhsT
            gf = gsb.tile([128, 2, GBLK], fp32, tag="gsb")
            for ki in range(2):
                tp = ps_et.tile([128, 128], fp32, tag="pset")
                nc.tensor.matmul(tp[:], g[:, ki * 128 : (ki + 1) * 128], eye[:],
                                 is_transpose=True, start=True, stop=True)
                nc.scalar.copy(gf[:, ki, :], tp[:])
            g_fm[j] = gf

        # ================= Phase B: timesteps =================
        for ts in range(NUM_TIMESTEPS):
            pooled_sb = [None] * nblk
            pooled_fm = [None] * nblk
            for j in range(nblk):
                bt = load_bcols(j)
                pooled = ps_pool.tile([GBLK, H], fp32, tag="pspool")
                for t, x_ap in x_stream(j):
                    e = make_E(bt, t)
                    # E_T (graph-major one-hot) via PE transpose
                    et = transpose_to_sbuf(e[:], etsb, "etsb")
                    # ctx = E @ G  (node-major)
                    ctxp = ps_ctx.tile([128, H], fp32, tag="psctx")
                    nc.tensor.matmul(ctxp[:], et[:], g_gm[j][:], start=True, stop=True)
                    # gate_in = x + ctx
                    gi = gisb.tile([128, H], fp32, tag="gisb")
                    nc.vector.tensor_add(gi[:], x_ap, ctxp[:])
                    # transpose gate_in -> (hi, n) chunks
                    git = ps_tr.tile([128, 2, 128], fp32, tag="pstr")
                    for ki in range(2):
                        nc.tensor.matmul(
                            git[:, ki, :], gi[:, ki * 128 : (ki + 1) * 128], eye[:],
                            is_transpose=True, start=True, stop=True,
                        )
                    gits = gisb.tile([128, 2, 128], fp32, tag="gits")
                    nc.scalar.copy(gits[:], git[:])
                    # h1^T = relu(W1 @ gate_in^T + b1): 2 M-chunks x 2 K-chunks
                    h1p = ps_h1.tile([128, 2, 128], fp32, tag="psh1")
                    for mo in range(2):
                        for ki in range(2):
                            nc.tensor.matmul(
                                h1p[:, mo, :],
                                w1t[ki][:, mo * 128 : (mo + 1) * 128],
                                gits[:, ki, :],
                                start=(ki == 0), stop=(ki == 1),
                            )
                    h1s = h1sb.tile([128, 2, 128], fp32, tag="h1sb")
                    for mo in range(2):
                        nc.scalar.activation(
                            h1s[:, mo, :], h1p[:, mo, :],
                            mybir.ActivationFunctionType.Relu, bias=b1[mo][:],
                        )
                    # gate = sigmoid(h1 @ w2 + b2) node-major (n,1)
                    gp = ps_gt.tile([128, 1], fp32, tag="psgt")
                    for mo in range(2):
                        nc.tensor.matmul(
                            gp[:], h1s[:, mo, :], w2[mo][:],
                            start=(mo == 0), stop=(mo == 1),
                        )
                    gt = gtsb.tile([128, 1], fp32, tag="gtsb")
                    nc.scalar.activation(
                        gt[:], gp[:], mybir.ActivationFunctionType.Sigmoid,
                        bias=b2col[:],
                    )
                    # wx = x * gate ; pooled += E.T @ wx
                    wx = wxsb.tile([128, H], fp32, tag="wxsb")
                    nc.vector.tensor_scalar(
                        wx[:], x_ap, gt[:], None, op0=mybir.AluOpType.mult
                    )
                    nc.tensor.matmul(
                        pooled[:], e[:], wx[:], start=(t == 0), stop=(t == NT - 1),
                        skip_group_check=True,
                    )
                ps = gsb.tile([GBLK, H], fp32, tag="poolsb")
                nc.vector.tensor_scalar(
                    ps[:], pooled[:], invc[j][:], None, op0=mybir.AluOpType.mult
                )
                pooled_sb[j] = ps
                pf = gsb.tile([128, 2, GBLK], fp32, tag="poolfm")
                for ki in range(2):
                    tp = ps_et.tile([128, 128], fp32, tag="pset")
                    nc.tensor.matmul(tp[:], ps[:, ki * 128 : (ki + 1) * 128], eye[:],
                                     is_transpose=True, start=True, stop=True)
                    nc.scalar.copy(pf[:, ki, :], tp[:])
                pooled_fm[j] = pf

            # ---- GRU cell per block (graph-major) ----
            for j in range(nblk):
                pf, gf, h_old = pooled_fm[j], g_fm[j], g_gm[j]

                def gru_mm(psum, wi, wh, bias_row, bcol0, bn):
                    # psum += pooled @ wih[:, cols] + h @ whh[:, cols] (+ bias)
                    mms = []
                    if wi is not None:
                        mms += [(pf[:, ki, :], wi[ki][:, bcol0 : bcol0 + bn])
                                for ki in range(2)]
                    if wh is not None:
                        mms += [(gf[:, ki, :], wh[ki][:, bcol0 : bcol0 + bn])
                                for ki in range(2)]
                    mms.append((ones_row[:], bias_row))
                    for i, (lhsT, rhs) in enumerate(mms):
                        nc.tensor.matmul(
                            psum[:], lhsT, rhs, start=(i == 0),
                            stop=(i == len(mms) - 1), skip_group_check=True,
                        )

                # r, z: gi+gh+b_ih+b_hh accumulated in one PSUM tile each
                rp = ps_ctx.tile([GBLK, H], fp32, tag="psctx")
                gru_mm(rp, wih, whh, brz[:, 0:H], 0, H)
                zp = ps_ctx.tile([GBLK, H], fp32, tag="psctx")
                gru_mm(zp, wih, whh, brz[:, H : 2 * H], H, H)
                inp_ = ps_tr.tile([GBLK, H], fp32, tag="pstr")
                gru_mm(inp_, wih, None, bin_[:], 2 * H, H)
                hnp = ps_h1.tile([GBLK, H], fp32, tag="psh1")
                gru_mm(hnp, None, whh, bhn[:], 2 * H, H)

                r = smallsb.tile([GBLK, H], fp32, tag="gru_r")
                nc.scalar.activation(r[:], rp[:], mybir.ActivationFunctionType.Sigmoid)
                z = smallsb.tile([GBLK, H], fp32, tag="gru_z")
                nc.scalar.activation(z[:], zp[:], mybir.ActivationFunctionType.Sigmoid)
                t1 = smallsb.tile([GBLK, H], fp32, tag="gru_t1")
                nc.vector.tensor_mul(t1[:], r[:], hnp[:])
                t2 = smallsb.tile([GBLK, H], fp32, tag="gru_t2")
                nc.vector.tensor_add(t2[:], t1[:], inp_[:])
                n = smallsb.tile([GBLK, H], fp32, tag="gru_n")
                nc.scalar.activation(n[:], t2[:], mybir.ActivationFunctionType.Tanh)
                # newh = n + z*(h - n), then relu
                t3 = smallsb.tile([GBLK, H], fp32, tag="gru_t3")
                nc.vector.tensor_sub(t3[:], h_old[:], n[:])
                t4 = smallsb.tile([GBLK, H], fp32, tag="gru_t4")
                nc.vector.tensor_mul(t4[:], z[:], t3[:])
                t5 = smallsb.tile([GBLK, H], fp32, tag="gru_t5")
                nc.vector.tensor_add(t5[:], n[:], t4[:])
                gnew = gsb.tile([GBLK, H], fp32, tag="gsb")
                nc.scalar.activation(gnew[:], t5[:],
                                     mybir.ActivationFunctionType.Relu)
                g_gm[j] = gnew
                if ts < NUM_TIMESTEPS - 1:
                    gfn = gsb.tile([128, 2, GBLK], fp32, tag="gsb")
                    for ki in range(2):
                        tp = ps_et.tile([128, 128], fp32, tag="pset")
                        nc.tensor.matmul(
                            tp[:], gnew[:, ki * 128 : (ki + 1) * 128], eye[:],
                            is_transpose=True, start=True, stop=True,
                        )
                        nc.scalar.copy(gfn[:, ki, :], tp[:])
                    g_fm[j] = gfn

        # ---- output ----
        for j in range(nblk):
            nc.sync.dma_start(out_d[j * GBLK : (j + 1) * GBLK, :], g_gm[j][:])

    nc.compile()
    return nc


def _prep_inputs(x, batch, counts, n_cores, nblk, NT=None):
    """Host-side shard + pad + layout. Returns (in_maps, NT)."""
    N = x.shape[0]
    G = n_cores * nblk * GBLK
    batch = np.asarray(batch).astype(np.int64)
    x = np.asarray(x, dtype=np.float32)

    # block boundaries: (n_cores*nblk + 1) node offsets at 128-graph edges
    edges = np.searchsorted(batch, np.arange(0, G + 1, GBLK))
    blk_cnt = np.diff(edges)  # nodes per 128-graph block
    if NT is None:
        NT = int(np.ceil(blk_cnt.max() / 128))
        NT = ((NT + CHUNK - 1) // CHUNK) * CHUNK  # multiple of CHUNK
    NTP = NT * 128

    invc_all = (1.0 / np.maximum(counts, 1.0)).astype(np.float32)  # (G,)

    # shared constant tensors filled by caller; here per-core data:
    per_core = []
    for k in range(n_cores):
        xk = np.zeros((nblk * NTP, H), dtype=np.float32)
        bcols = np.full((nblk, 128, NT), -1.0, dtype=np.float32)
        for j in range(nblk):
            bi = k * nblk + j
            lo, hi = edges[bi], edges[bi + 1]
            cnt = hi - lo
            xk[j * NTP : j * NTP + cnt] = x[lo:hi]
            bl = (batch[lo:hi] - (bi * GBLK)).astype(np.float32)
            blp = np.full(NTP, -1.0, dtype=np.float32)
            blp[:cnt] = bl
            bcols[j] = blp.reshape(NT, 128).T
        invc = invc_all[k * nblk * GBLK : (k + 1) * nblk * GBLK].reshape(
            nblk, GBLK, 1
        )
        per_core.append({"xk": xk, "bcols": bcols, "invc": np.ascontiguousarray(invc)})
    return per_core, NT


def _const_inputs(gate_w1, gate_b1, gate_w2, gate_b2, gru_w_ih, gru_w_hh,
                  gru_b_ih, gru_b_hh):
    f = np.float32
    c = {}
    c["w1t"] = np.ascontiguousarray(np.asarray(gate_w1, f).T.reshape(2, 128, H))
    c["b1"] = np.asarray(gate_b1, f).reshape(2, 128, 1)
    c["w2"] = np.ascontiguousarray(np.asarray(gate_w2, f)[0].reshape(2, 128, 1))
    c["b2"] = np.asarray(gate_b2, f).reshape(1, 1)
    c["wih_t"] = np.ascontiguousarray(np.asarray(gru_w_ih, f).T).reshape(2, 128, 3 * H)
    c["whh_t"] = np.ascontiguousarray(np.asarray(gru_w_hh, f).T).reshape(2, 128, 3 * H)
    bih = np.asarray(gru_b_ih, f)
    bhh = np.asarray(gru_b_hh, f)
    c["bsum_rz"] = (bih[: 2 * H] + bhh[: 2 * H]).reshape(1, 2 * H)
    c["bihn"] = bih[2 * H :].reshape(1, H)
    c["bhhn"] = bhh[2 * H :].reshape(1, H)
    c["iota_row"] = np.tile(np.arange(128, dtype=f), (128, 1))
    c["eye128"] = np.eye(128, dtype=f)
    return c


_CACHE = {}


def run(x, gate_w1, gate_b1, gate_w2, gate_b2, gru_w_ih, gru_w_hh, gru_b_ih,
        gru_b_hh, batch, num_graphs, n_cores=8, nblk=NBLK, trace=False,
        use_sim=False):
    from concourse.bass_utils import run_bass_kernel_spmd

    batch = np.asarray(batch).astype(np.int64)
    G = n_cores * nblk * GBLK
    counts = np.bincount(batch, minlength=G).astype(np.float32)
    per_core, NT = _prep_inputs(x, batch, counts, n_cores, nblk)
    consts = _const_inputs(gate_w1, gate_b1, gate_w2, gate_b2, gru_w_ih,
                           gru_w_hh, gru_b_ih, gru_b_hh)
    in_maps = [{**consts, **pc} for pc in per_core]

    key = (NT, nblk, n_cores)
    if key not in _CACHE:
        _CACHE[key] = _build_program(NT, nblk=nblk)
    nc = _CACHE[key]

    if use_sim:
        from concourse.bass_interp import CoreSim

        outs = []
        for k in range(n_cores):
            sim = CoreSim(nc)
            for name, arr in in_maps[k].items():
                sim.tensor(name)[:] = arr
            sim.simulate()
            outs.append(np.array(sim.tensor("out")))
        return np.concatenate(outs, axis=0), None

    res = run_bass_kernel_spmd(nc, in_maps, core_ids=list(range(n_cores)),
                               trace=trace)
    out = np.concatenate([res.results[k]["out"] for k in range(n_cores)], axis=0)
    return out, res


def kernel(**inputs):
    out, _ = run(**inputs)
    return out


# revision 8
# speedup vs baseline: 1.6404x; 1.6404x over previous
"""Trainium2 Bass kernel for AttentiveGraphPooling (gnn_message_passing).

Strategy: shard the 4096 graphs across 8 cores (512 graphs each). batch is
sorted, so each core owns a contiguous node range covering whole graphs ->
pooling / gather / GRU are all core-local, no collectives needed.

Per core, graphs go in 4 blocks of 128. Each block's node features (bf16) are
DMA'd once and stay resident in SBUF for all three passes (init mean-pool +
2 timesteps). A per-tile one-hot E (node x local-graph, built by iota/is_equal
on the vector engine) turns the segment mean-pool (E.T @ x, PSUM-accumulated)
and the graph-context gather (E @ G) into TensorEngine matmuls; E.T for the
gather is formed once per block on the PE and kept resident. The gate MLP runs
in bf16 with weights stationary; the GRU cell runs per graph-block in f32 with
biases folded in via K=1 matmuls.
"""

import os
import sys

import numpy as np

sys.path.insert(0, "/opt/trn_rl_repo")

H = 256
NBLK = 4  # graph blocks per core
GBLK = 128  # graphs per block
NUM_TIMESTEPS = 2
LCHUNK = 16  # node tiles per resident-load DMA


def _build_program(NT, nblk=NBLK):
    """Build the single-core SPMD Bass program. NT = node tiles per block."""
    from contextlib import ExitStack

    import concourse.bass as bass
    import concourse.tile as tile
    from concourse import bacc, mybir

    fp32 = mybir.dt.float32
    bf16 = mybir.dt.bfloat16

    NTP = NT * 128  # padded nodes per block

    nc = bacc.Bacc("TRN2", target_bir_lowering=False, debug=False)

    # ---- DRAM parameters (per-core inputs) ----
    x_d = nc.dram_tensor("xk", [nblk * NTP, H], bf16, kind="ExternalInput")
    bcols_d = nc.dram_tensor("bcols", [nblk, 128, NT], fp32, kind="ExternalInput")
    invc_d = nc.dram_tensor("invc", [nblk, GBLK, 1], fp32, kind="ExternalInput")
    w1t_d = nc.dram_tensor("w1t", [2, 128, H], bf16, kind="ExternalInput")
    b1_d = nc.dram_tensor("b1", [2, 128, 1], fp32, kind="ExternalInput")
    w2_d = nc.dram_tensor("w2", [2, 128, 1], bf16, kind="ExternalInput")
    wih_d = nc.dram_tensor("wih_t", [2, 128, 3 * H], bf16, kind="ExternalInput")
    whh_d = nc.dram_tensor("whh_t", [2, 128, 3 * H], bf16, kind="ExternalInput")
    brz_d = nc.dram_tensor("bsum_rz", [1, 2 * H], fp32, kind="ExternalInput")
    bin_d = nc.dram_tensor("bihn", [1, H], fp32, kind="ExternalInput")
    bhn_d = nc.dram_tensor("bhhn", [1, H], fp32, kind="ExternalInput")
    b2_d = nc.dram_tensor("b2", [1, 1], fp32, kind="ExternalInput")
    iota_d = nc.dram_tensor("iota_row", [128, 128], fp32, kind="ExternalInput")
    eye_d = nc.dram_tensor("eye128", [128, 128], fp32, kind="ExternalInput")
    eyeb_d = nc.dram_tensor("eye128b", [128, 128], bf16, kind="ExternalInput")
    out_d = nc.dram_tensor("out", [nblk * GBLK, H], fp32, kind="ExternalOutput")

    with tile.TileContext(nc) as tc, ExitStack() as ctx:
        ep = ctx.enter_context  # shorthand

        const = ep(tc.tile_pool(name="const", bufs=1))
        xres = ep(tc.tile_pool(name="xres", bufs=2))
        etres = ep(tc.tile_pool(name="etres", bufs=1))
        bpool = ep(tc.tile_pool(name="bcols", bufs=2))
        epool = ep(tc.tile_pool(name="eoh", bufs=3))
        gisb = ep(tc.tile_pool(name="gisb", bufs=3))
        h1sb = ep(tc.tile_pool(name="h1sb", bufs=2))
        wxsb = ep(tc.tile_pool(name="wxsb", bufs=2))
        gtsb = ep(tc.tile_pool(name="gtsb", bufs=2))
        gsb = ep(tc.tile_pool(name="gsb", bufs=3))
        smallsb = ep(tc.tile_pool(name="smallsb", bufs=2))

        ps_pool = ep(tc.tile_pool(name="pspool", bufs=2, space="PSUM"))
        ps_et = ep(tc.tile_pool(name="pset", bufs=1, space="PSUM"))
        ps_ctx = ep(tc.tile_pool(name="psctx", bufs=2, space="PSUM"))
        ps_tr = ep(tc.tile_pool(name="pstr", bufs=2, space="PSUM"))
        ps_h1 = ep(tc.tile_pool(name="psh1", bufs=1, space="PSUM"))

        # ---- load constants ----
        def cload(shape, src, tag, dt=fp32):
            t = const.tile(shape, dt, tag=tag)
            nc.sync.dma_start(t[:], src)
            return t

        iota_row = cload([128, 128], iota_d[:], "c_iota")
        eye = cload([128, 128], eye_d[:], "c_eye")
        eyeb = cload([128, 128], eyeb_d[:], "c_eyeb", bf16)
        w1t = [cload([128, H], w1t_d[k], f"c_w1t{k}", bf16) for k in range(2)]
        b1 = [cload([128, 1], b1_d[k], f"c_b1{k}") for k in range(2)]
        w2 = [cload([128, 1], w2_d[k], f"c_w2{k}", bf16) for k in range(2)]
        wih = [cload([128, 3 * H], wih_d[k], f"c_wih{k}", bf16) for k in range(2)]
        whh = [cload([128, 3 * H], whh_d[k], f"c_whh{k}", bf16) for k in range(2)]
        brz = cload([1, 2 * H], brz_d[:], "c_brz")
        bin_ = cload([1, H], bin_d[:], "c_bin")
        bhn = cload([1, H], bhn_d[:], "c_bhn")
        b2t = cload([1, 1], b2_d[:], "c_b2")
        invc = [cload([GBLK, 1], invc_d[j], f"c_invc{j}") for j in range(nblk)]
        ones_row = const.tile([1, 128], fp32)
        nc.vector.memset(ones_row[:], 1.0)
        ones_rowb = const.tile([1, 128], bf16)
        nc.vector.memset(ones_rowb[:], 1.0)
        # broadcast b2 scalar to all 128 partitions (K=1 matmul), keep in SBUF
        b2ps = ps_et.tile([128, 1], fp32, tag="pset")
        nc.tensor.matmul(b2ps[:], ones_row[:], b2t[:], start=True, stop=True)
        b2col = const.tile([128, 1], fp32)
        nc.scalar.copy(b2col[:], b2ps[:])

        def make_E(bt, t):
            e = epool.tile([128, 128], bf16, tag="eoh")
            nc.vector.tensor_scalar(
                e[:], iota_row[:], bt[:, t : t + 1], None, op0=mybir.AluOpType.is_equal
            )
            return e

        def fm_copy(g_ap, pool, tag, dt, engine):
            """(128,256) graph-major -> feature-major (128,2,128) via PE."""
            gf = pool.tile([128, 2, GBLK], dt, tag=tag)
            for ki in range(2):
                tp = ps_et.tile([128, 128], fp32, tag="pset")
                nc.tensor.matmul(tp[:], g_ap[:, ki * 128 : (ki + 1) * 128], eye[:],
                                 is_transpose=True, start=True, stop=True)
                engine(gf[:, ki, :], tp[:])
            return gf

        out_tiles = [None] * nblk

        for j in range(nblk):
            # ---- resident x for this block ----
            xj = xres.tile([128, NT, H], bf16, tag="xres")
            for c0 in range(0, NT, LCHUNK):
                cn = min(LCHUNK, NT - c0)
                base = j * NTP + c0 * 128
                src = x_d[base : base + cn * 128, :].rearrange(
                    "(c p) h -> p c h", p=128
                )
                nc.sync.dma_start(xj[:, c0 : c0 + cn, :], src)
            bt = bpool.tile([128, NT], fp32, tag="bcols")
            nc.sync.dma_start(bt[:], bcols_d[j])

            # ---- phase A: initial mean pool ----
            pooled = ps_pool.tile([GBLK, H], fp32, tag="pspool")
            for t in range(NT):
                e = make_E(bt, t)
                nc.tensor.matmul(
                    pooled[:], e[:], xj[:, t, :], start=(t == 0), stop=(t == NT - 1),
                    skip_group_check=True,
                )
            g_gm = gsb.tile([GBLK, H], fp32, tag="gsb")
            nc.vector.tensor_scalar(
                g_gm[:], pooled[:], invc[j][:], None, op0=mybir.AluOpType.mult
            )
            g_bf = gsb.tile([GBLK, H], bf16, tag="gbf")
            nc.scalar.copy(g_bf[:], g_gm[:])
            g_fm = fm_copy(g_gm[:], gsb, "gfm", bf16, nc.scalar.copy)

            etj = etres.tile([128, NT, 128], bf16, tag="etres")

            # ---- timesteps ----
            for ts in range(NUM_TIMESTEPS):
                pooled = ps_pool.tile([GBLK, H], fp32, tag="pspool")
                for t in range(NT):
                    e = make_E(bt, t)
                    if ts == 0:
                        # build resident E^T (graph-major one-hot) once
                        tp = ps_et.tile([128, 128], bf16, tag="pset")
                        nc.tensor.matmul(tp[:], e[:], eyeb[:], is_transpose=True,
                                         start=True, stop=True)
                        nc.scalar.copy(etj[:, t, :], tp[:])
                    # ctx = E @ G  (node-major)
                    ctxp = ps_ctx.tile([128, H], fp32, tag="psctx")
                    nc.tensor.matmul(ctxp[:], etj[:, t, :], g_bf[:],
                                     start=True, stop=True)
                    # gate_in = x + ctx  (bf16)
                    gi = gisb.tile([128, H], bf16, tag="gisb")
                    nc.vector.tensor_add(gi[:], xj[:, t, :], ctxp[:])
                    # transpose gate_in -> (hi, n) chunks
                    git = ps_tr.tile([128, 2, 128], bf16, tag="pstr")
                    for ki in range(2):
                        nc.tensor.matmul(
                            git[:, ki, :], gi[:, ki * 128 : (ki + 1) * 128], eyeb[:],
                            is_transpose=True, start=True, stop=True,
                        )
                    gits = gisb.tile([128, 2, 128], bf16, tag="gits")
                    nc.vector.tensor_copy(gits[:], git[:])
                    # h1^T = relu(W1 @ gate_in^T + b1): 2 M-chunks x 2 K-chunks
                    h1p = ps_h1.tile([128, 2, 128], fp32, tag="psh1")
                    for mo in range(2):
                        for ki in range(2):
                            nc.tensor.matmul(
                                h1p[:, mo, :],
                                w1t[ki][:, mo * 128 : (mo + 1) * 128],
                                gits[:, ki, :],
                                start=(ki == 0), stop=(ki == 1),
                            )
                    h1s = h1sb.tile([128, 2, 128], bf16, tag="h1sb")
                    for mo in range(2):
                        nc.scalar.activation(
                            h1s[:, mo, :], h1p[:, mo, :],
                            mybir.ActivationFunctionType.Relu, bias=b1[mo][:],
                        )
                    # gate = sigmoid(h1 @ w2 + b2) node-major (n,1)
                    gp = ps_tr.tile([128, 1], fp32, tag="pstr")
                    for mo in range(2):
                        nc.tensor.matmul(
                            gp[:], h1s[:, mo, :], w2[mo][:],
                            start=(mo == 0), stop=(mo == 1),
                        )
                    gt = gtsb.tile([128, 1], fp32, tag="gtsb")
                    nc.scalar.activation(
                        gt[:], gp[:], mybir.ActivationFunctionType.Sigmoid,
                        bias=b2col[:],
                    )
                    # wx = x * gate ; pooled += E.T @ wx
                    wx = wxsb.tile([128, H], bf16, tag="wxsb")
                    nc.vector.tensor_scalar(
                        wx[:], xj[:, t, :], gt[:], None, op0=mybir.AluOpType.mult
                    )
                    nc.tensor.matmul(
                        pooled[:], e[:], wx[:], start=(t == 0), stop=(t == NT - 1),
                        skip_group_check=True,
                    )
                ps = gsb.tile([GBLK, H], fp32, tag="poolsb")
                nc.vector.tensor_scalar(
                    ps[:], pooled[:], invc[j][:], None, op0=mybir.AluOpType.mult
                )
                pf = fm_copy(ps[:], gsb, "poolfm", bf16, nc.scalar.copy)

                # ---- GRU cell (graph-major) ----
                gf, h_old = g_fm, g_gm

                def gru_mm(psum, wi, wh, bias_row, bcol0, bn):
                    mms = []
                    if wi is not None:
                        mms += [(pf[:, ki, :], wi[ki][:, bcol0 : bcol0 + bn], ones_rowb)
                                for ki in range(2)]
                    if wh is not None:
                        mms += [(gf[:, ki, :], wh[ki][:, bcol0 : bcol0 + bn], ones_rowb)
                                for ki in range(2)]
                    for i, (lhsT, rhs, _) in enumerate(mms):
                        nc.tensor.matmul(
                            psum[:], lhsT, rhs, start=(i == 0), stop=False,
                            skip_group_check=True,
                        )
                    nc.tensor.matmul(
                        psum[:], ones_row[:], bias_row, start=False, stop=True,
                        skip_group_check=True,
                    )

                rp = ps_ctx.tile([GBLK, H], fp32, tag="psctx")
                gru_mm(rp, wih, whh, brz[:, 0:H], 0, H)
                zp = ps_ctx.tile([GBLK, H], fp32, tag="psctx")
                gru_mm(zp, wih, whh, brz[:, H : 2 * H], H, H)
                inp_ = ps_tr.tile([GBLK, H], fp32, tag="pstr")
                gru_mm(inp_, wih, None, bin_[:], 2 * H, H)
                hnp = ps_h1.tile([GBLK, H], fp32, tag="psh1")
                gru_mm(hnp, None, whh, bhn[:], 2 * H, H)

                r = smallsb.tile([GBLK, H], fp32, tag="gru_r")
                nc.scalar.activation(r[:], rp[:], mybir.ActivationFunctionType.Sigmoid)
                z = smallsb.tile([GBLK, H], fp32, tag="gru_z")
                nc.scalar.activation(z[:], zp[:], mybir.ActivationFunctionType.Sigmoid)
                t1 = smallsb.tile([GBLK, H], fp32, tag="gru_t1")
                nc.vector.tensor_mul(t1[:], r[:], hnp[:])
                t2 = smallsb.tile([GBLK, H], fp32, tag="gru_t2")
                nc.vector.tensor_add(t2[:], t1[:], inp_[:])
                n = smallsb.tile([GBLK, H], fp32, tag="gru_n")
                nc.scalar.activation(n[:], t2[:], mybir.ActivationFunctionType.Tanh)
                t3 = smallsb.tile([GBLK, H], fp32, tag="gru_t3")
                nc.vector.tensor_sub(t3[:], h_old[:], n[:])
                t4 = smallsb.tile([GBLK, H], fp32, tag="gru_t4")
                nc.vector.tensor_mul(t4[:], z[:], t3[:])
                t5 = smallsb.tile([GBLK, H], fp32, tag="gru_t5")
                nc.vector.tensor_add(t5[:], n[:], t4[:])
                g_gm = gsb.tile([GBLK, H], fp32, tag="gsb")
                nc.scalar.activation(g_gm[:], t5[:],
                                     mybir.ActivationFunctionType.Relu)
                if ts < NUM_TIMESTEPS - 1:
                    g_bf = gsb.tile([GBLK, H], bf16, tag="gbf")
                    nc.scalar.copy(g_bf[:], g_gm[:])
                    g_fm = fm_copy(g_gm[:], gsb, "gfm", bf16, nc.scalar.copy)

            nc.sync.dma_start(out_d[j * GBLK : (j + 1) * GBLK, :], g_gm[:])
            out_tiles[j] = g_gm

    nc.compile()
    return nc


def _prep_inputs(x, batch, counts, n_cores, nblk, NT=None):
    """Host-side shard + pad + layout. Returns (per_core, NT)."""
    import ml_dtypes

    G = n_cores * nblk * GBLK
    batch = np.asarray(batch).astype(np.int64)
    x = np.asarray(x, dtype=np.float32)

    edges = np.searchsorted(batch, np.arange(0, G + 1, GBLK))
    blk_cnt = np.diff(edges)
    if NT is None:
        NT = int(np.ceil(blk_cnt.max() / 128))
        NT = ((NT + LCHUNK - 1) // LCHUNK) * LCHUNK
    NTP = NT * 128

    invc_all = (1.0 / np.maximum(counts, 1.0)).astype(np.float32)

    xb = x.astype(ml_dtypes.bfloat16)
    per_core = []
    for k in range(n_cores):
        xk = np.zeros((nblk * NTP, H), dtype=ml_dtypes.bfloat16)
        bcols = np.full((nblk, 128, NT), -1.0, dtype=np.float32)
        for j in range(nblk):
            bi = k * nblk + j
            lo, hi = edges[bi], edges[bi + 1]
            cnt = hi - lo
            xk[j * NTP : j * NTP + cnt] = xb[lo:hi]
            blp = np.full(NTP, -1.0, dtype=np.float32)
            blp[:cnt] = (batch[lo:hi] - (bi * GBLK)).astype(np.float32)
            bcols[j] = blp.reshape(NT, 128).T
        invc = invc_all[k * nblk * GBLK : (k + 1) * nblk * GBLK].reshape(
            nblk, GBLK, 1
        )
        per_core.append({"xk": xk, "bcols": bcols, "invc": np.ascontiguousarray(invc)})
    return per_core, NT


def _const_inputs(gate_w1, gate_b1, gate_w2, gate_b2, gru_w_ih, gru_w_hh,
                  gru_b_ih, gru_b_hh):
    import ml_dtypes

    f = np.float32
    bf = ml_dtypes.bfloat16
    c = {}
    c["w1t"] = np.ascontiguousarray(
        np.asarray(gate_w1, f).T.reshape(2, 128, H)).astype(bf)
    c["b1"] = np.asarray(gate_b1, f).reshape(2, 128, 1)
    c["w2"] = np.ascontiguousarray(
        np.asarray(gate_w2, f)[0].reshape(2, 128, 1)).astype(bf)
    c["b2"] = np.asarray(gate_b2, f).reshape(1, 1)
    c["wih_t"] = np.ascontiguousarray(
        np.asarray(gru_w_ih, f).T).reshape(2, 128, 3 * H).astype(bf)
    c["whh_t"] = np.ascontiguousarray(
        np.asarray(gru_w_hh, f).T).reshape(2, 128, 3 * H).astype(bf)
    bih = np.asarray(gru_b_ih, f)
    bhh = np.asarray(gru_b_hh, f)
    c["bsum_rz"] = (bih[: 2 * H] + bhh[: 2 * H]).reshape(1, 2 * H)
    c["bihn"] = bih[2 * H :].reshape(1, H)
    c["bhhn"] = bhh[2 * H :].reshape(1, H)
    c["iota_row"] = np.tile(np.arange(128, dtype=f), (128, 1))
    c["eye128"] = np.eye(128, dtype=f)
    c["eye128b"] = np.eye(128, dtype=f).astype(bf)
    return c


_CACHE = {}


def run(x, gate_w1, gate_b1, gate_w2, gate_b2, gru_w_ih, gru_w_hh, gru_b_ih,
        gru_b_hh, batch, num_graphs, n_cores=8, nblk=NBLK, trace=False,
        use_sim=False):
    from concourse.bass_utils import run_bass_kernel_spmd

    batch = np.asarray(batch).astype(np.int64)
    G = n_cores * nblk * GBLK
    counts = np.bincount(batch, minlength=G).astype(np.float32)
    per_core, NT = _prep_inputs(x, batch, counts, n_cores, nblk)
    consts = _const_inputs(gate_w1, gate_b1, gate_w2, gate_b2, gru_w_ih,
                           gru_w_hh, gru_b_ih, gru_b_hh)
    in_maps = [{**consts, **pc} for pc in per_core]

    key = (NT, nblk, n_cores)
    if key not in _CACHE:
        _CACHE[key] = _build_program(NT, nblk=nblk)
    nc = _CACHE[key]

    if use_sim:
        from concourse.bass_interp import CoreSim

        outs = []
        for k in range(n_cores):
            sim = CoreSim(nc)
            for name, arr in in_maps[k].items():
                sim.tensor(name)[:] = arr
            sim.simulate()
            outs.append(np.array(sim.tensor("out")))
        return np.concatenate(outs, axis=0), None

    res = run_bass_kernel_spmd(nc, in_maps, core_ids=list(range(n_cores)),
                               trace=trace)
    out = np.concatenate([res.results[k]["out"] for k in range(n_cores)], axis=0)
    return out, res


def kernel(**inputs):
    out, _ = run(**inputs)
    return out


# revision 12
# speedup vs baseline: 2.0650x; 1.2588x over previous
"""Trainium2 Bass kernel for AttentiveGraphPooling (gnn_message_passing).

Strategy: shard the 4096 graphs across 8 cores (512 graphs each). batch is
sorted, so each core owns a contiguous node range covering whole graphs ->
pooling / gather / GRU are all core-local, no collectives needed.

Per core, graphs go in 4 blocks of 128. Each block's node features (bf16) are
DMA'd once and stay resident in SBUF for all three passes (init mean-pool +
2 timesteps). A per-tile one-hot E (node x local-graph, built by iota/is_equal
on the vector engine) turns the segment mean-pool (E.T @ x, PSUM-accumulated)
and the per-node context gather into TensorEngine matmuls; E.T is formed once
per block on the PE and kept resident.

The gate MLP uses distributivity to stay node-major with no per-node add:
  h1[n] = relu(W1 @ (x_n + g_b(n)) + b1) = relu(x_n @ W1^T + GW1[b(n)])
where GW1 = G @ W1^T + b1 is built once per block/timestep (graph-level).
x^T tiles feed W1 matmuls as stationary operands; the E^T gather of GW1
accumulates into the same PSUM tile. The scalar gate = sigmoid(h1 . w2 + b2)
is a vector-engine tensor_tensor_reduce (w2 pre-broadcast, b2 as reduce
init), avoiding N=1 matmuls. The GRU runs per graph-block with biases folded
in via K=1 matmuls.
"""

import os
import sys

import numpy as np

sys.path.insert(0, "/opt/trn_rl_repo")

H = 256
NBLK = 4  # graph blocks per core
GBLK = 128  # graphs per block
NUM_TIMESTEPS = 2
LCHUNK = 16  # node tiles per resident-load DMA


def _build_program(NT, nblk=NBLK):
    """Build the single-core SPMD Bass program. NT = node tiles per block."""
    from contextlib import ExitStack

    import concourse.bass as bass
    import concourse.tile as tile
    from concourse import bacc, mybir

    fp32 = mybir.dt.float32
    bf16 = mybir.dt.bfloat16

    NTP = NT * 128  # padded nodes per block

    nc = bacc.Bacc("TRN2", target_bir_lowering=False, debug=False)

    # ---- DRAM parameters (per-core inputs) ----
    x_d = nc.dram_tensor("xk", [nblk * NTP, H], bf16, kind="ExternalInput")
    bcols_d = nc.dram_tensor("bcols", [nblk, 128, NT], fp32, kind="ExternalInput")
    invc_d = nc.dram_tensor("invc", [nblk, GBLK, 1], fp32, kind="ExternalInput")
    w1t_d = nc.dram_tensor("w1t", [2, 128, H], bf16, kind="ExternalInput")
    b1r_d = nc.dram_tensor("b1row", [1, H], fp32, kind="ExternalInput")
    w2bc_d = nc.dram_tensor("w2bc", [128, H], bf16, kind="ExternalInput")
    b2c_d = nc.dram_tensor("b2col", [128, 1], fp32, kind="ExternalInput")
    wih_d = nc.dram_tensor("wih_t", [2, 128, 3 * H], bf16, kind="ExternalInput")
    whh_d = nc.dram_tensor("whh_t", [2, 128, 3 * H], bf16, kind="ExternalInput")
    brz_d = nc.dram_tensor("bsum_rz", [1, 2 * H], fp32, kind="ExternalInput")
    bin_d = nc.dram_tensor("bihn", [1, H], fp32, kind="ExternalInput")
    bhn_d = nc.dram_tensor("bhhn", [1, H], fp32, kind="ExternalInput")
    iota_d = nc.dram_tensor("iota_row", [128, 128], fp32, kind="ExternalInput")
    eye_d = nc.dram_tensor("eye128", [128, 128], fp32, kind="ExternalInput")
    eyeb_d = nc.dram_tensor("eye128b", [128, 128], bf16, kind="ExternalInput")
    out_d = nc.dram_tensor("out", [nblk * GBLK, H], fp32, kind="ExternalOutput")

    with tile.TileContext(nc) as tc, ExitStack() as ctx:
        ep = ctx.enter_context  # shorthand

        const = ep(tc.tile_pool(name="const", bufs=1))
        xres = ep(tc.tile_pool(name="xres", bufs=2))
        etres = ep(tc.tile_pool(name="etres", bufs=1))
        bpool = ep(tc.tile_pool(name="bcols", bufs=2))
        epool = ep(tc.tile_pool(name="eoh", bufs=4))
        xtsb = ep(tc.tile_pool(name="xtsb", bufs=4))
        h1sb = ep(tc.tile_pool(name="h1sb", bufs=3))
        trsh = ep(tc.tile_pool(name="trsh", bufs=2))
        wxsb = ep(tc.tile_pool(name="wxsb", bufs=3))
        gtsb = ep(tc.tile_pool(name="gtsb", bufs=3))
        gsb = ep(tc.tile_pool(name="gsb", bufs=3))
        smallsb = ep(tc.tile_pool(name="smallsb", bufs=2))

        ps_pool = ep(tc.tile_pool(name="pspool", bufs=2, space="PSUM"))
        ps_tr = ep(tc.tile_pool(name="pstr", bufs=2, space="PSUM"))
        ps_h1 = ep(tc.tile_pool(name="psh1", bufs=2, space="PSUM"))
        ps_et = ep(tc.tile_pool(name="pset", bufs=2, space="PSUM"))

        # ---- load constants ----
        def cload(shape, src, tag, dt=fp32):
            t = const.tile(shape, dt, tag=tag)
            nc.sync.dma_start(t[:], src)
            return t

        iota_row = cload([128, 128], iota_d[:], "c_iota")
        eye = cload([128, 128], eye_d[:], "c_eye")
        eyeb = cload([128, 128], eyeb_d[:], "c_eyeb", bf16)
        w1t = [cload([128, H], w1t_d[k], f"c_w1t{k}", bf16) for k in range(2)]
        b1row = cload([1, H], b1r_d[:], "c_b1r")
        w2bc = cload([128, H], w2bc_d[:], "c_w2bc", bf16)
        b2col = cload([128, 1], b2c_d[:], "c_b2c")
        wih = [cload([128, 3 * H], wih_d[k], f"c_wih{k}", bf16) for k in range(2)]
        whh = [cload([128, 3 * H], whh_d[k], f"c_whh{k}", bf16) for k in range(2)]
        brz = cload([1, 2 * H], brz_d[:], "c_brz")
        bin_ = cload([1, H], bin_d[:], "c_bin")
        bhn = cload([1, H], bhn_d[:], "c_bhn")
        invc = [cload([GBLK, 1], invc_d[j], f"c_invc{j}") for j in range(nblk)]
        ones_row = const.tile([1, 128], fp32)
        nc.vector.memset(ones_row[:], 1.0)

        def make_E(bt, t):
            e = epool.tile([128, 128], bf16, tag="eoh")
            nc.vector.tensor_scalar(
                e[:], iota_row[:], bt[:, t : t + 1], None, op0=mybir.AluOpType.is_equal
            )
            return e

        def fm_copy(g_ap, pool, tag, dt):
            """(128,256) graph-major -> feature-major (128,2,128) via PE."""
            gf = pool.tile([128, 2, GBLK], dt, tag=tag)
            for ki in range(2):
                tp = ps_et.tile([128, 128], fp32, tag="pset")
                nc.tensor.matmul(tp[:], g_ap[:, ki * 128 : (ki + 1) * 128], eye[:],
                                 is_transpose=True, start=True, stop=True)
                nc.scalar.copy(gf[:, ki, :], tp[:])
            return gf

        for j in range(nblk):
            # ---- resident x for this block ----
            xj = xres.tile([128, NT, H], bf16, tag="xres")
            for c0 in range(0, NT, LCHUNK):
                cn = min(LCHUNK, NT - c0)
                base = j * NTP + c0 * 128
                src = x_d[base : base + cn * 128, :].rearrange(
                    "(c p) h -> p c h", p=128
                )
                nc.sync.dma_start(xj[:, c0 : c0 + cn, :], src)
            bt = bpool.tile([128, NT], fp32, tag="bcols")
            nc.sync.dma_start(bt[:], bcols_d[j])

            # ---- phase A: initial mean pool ----
            pooled = ps_pool.tile([GBLK, H], fp32, tag="pspool")
            for t in range(NT):
                e = make_E(bt, t)
                nc.tensor.matmul(
                    pooled[:], e[:], xj[:, t, :], start=(t == 0), stop=(t == NT - 1),
                    skip_group_check=True,
                )
            g_gm = gsb.tile([GBLK, H], fp32, tag="gsb")
            nc.vector.tensor_scalar(
                g_gm[:], pooled[:], invc[j][:], None, op0=mybir.AluOpType.mult
            )
            g_fm = fm_copy(g_gm[:], gsb, "gfm", bf16)

            etj = etres.tile([128, NT, 128], bf16, tag="etres")

            # ---- timesteps ----
            for ts in range(NUM_TIMESTEPS):
                # GW1 = G @ W1^T + b1  (graph-level, bf16)
                gw1p = ps_et.tile([GBLK, H], fp32, tag="pset")
                for ki in range(2):
                    nc.tensor.matmul(gw1p[:], g_fm[:, ki, :], w1t[ki][:],
                                     start=(ki == 0), stop=False,
                                     skip_group_check=True)
                nc.tensor.matmul(gw1p[:], ones_row[:], b1row[:],
                                 start=False, stop=True, skip_group_check=True)
                gw1 = gsb.tile([GBLK, H], bf16, tag="gw1")
                nc.scalar.copy(gw1[:], gw1p[:])

                pooled = ps_pool.tile([GBLK, H], fp32, tag="pspool")
                for t in range(NT):
                    e = make_E(bt, t)
                    if ts == 0:
                        # build resident E^T (graph-major one-hot) once
                        tp = ps_et.tile([128, 128], bf16, tag="pset")
                        nc.tensor.matmul(tp[:], e[:], eyeb[:], is_transpose=True,
                                         start=True, stop=True)
                        nc.vector.tensor_copy(etj[:, t, :], tp[:])
                    # x^T chunks (hi, n) via PE transpose
                    xtp = ps_tr.tile([128, 2, 128], bf16, tag="pstr")
                    for ki in range(2):
                        nc.tensor.matmul(
                            xtp[:, ki, :],
                            xj[:, t, ki * 128 : (ki + 1) * 128], eyeb[:],
                            is_transpose=True, start=True, stop=True,
                        )
                    xt = xtsb.tile([128, 2, 128], bf16, tag="xtsb")
                    nc.scalar.copy(xt[:], xtp[:])
                    # h1 = relu(x @ W1^T + GW1[b])  node-major, one PSUM accum
                    h1p = ps_h1.tile([128, H], fp32, tag="psh1")
                    for ki in range(2):
                        nc.tensor.matmul(
                            h1p[:], xt[:, ki, :], w1t[ki][:],
                            start=(ki == 0), stop=False,
                        )
                    nc.tensor.matmul(h1p[:], etj[:, t, :], gw1[:],
                                     start=False, stop=True)
                    h1s = h1sb.tile([128, H], bf16, tag="h1sb")
                    nc.vector.tensor_scalar(
                        h1s[:], h1p[:], 0.0, None, op0=mybir.AluOpType.max
                    )
                    # gate = sigmoid(h1 . w2 + b2) via DVE mult + reduce
                    trash = trsh.tile([128, H], bf16, tag="trsh")
                    nc.vector.tensor_mul(trash[:], h1s[:], w2bc[:])
                    gpre = gtsb.tile([128, 1], fp32, tag="gpre")
                    nc.vector.reduce_sum(gpre[:], trash[:], mybir.AxisListType.X)
                    gt = gtsb.tile([128, 1], fp32, tag="gtsb")
                    nc.scalar.activation(
                        gt[:], gpre[:], mybir.ActivationFunctionType.Sigmoid,
                        bias=b2col[:],
                    )
                    # wx = x * gate ; pooled += E.T @ wx
                    wx = wxsb.tile([128, H], bf16, tag="wxsb")
                    nc.vector.tensor_scalar(
                        wx[:], xj[:, t, :], gt[:], None, op0=mybir.AluOpType.mult
                    )
                    nc.tensor.matmul(
                        pooled[:], e[:], wx[:], start=(t == 0), stop=(t == NT - 1),
                        skip_group_check=True,
                    )
                ps = gsb.tile([GBLK, H], fp32, tag="poolsb")
                nc.vector.tensor_scalar(
                    ps[:], pooled[:], invc[j][:], None, op0=mybir.AluOpType.mult
                )
                pf = fm_copy(ps[:], gsb, "poolfm", bf16)

                # ---- GRU cell (graph-major) ----
                gf, h_old = g_fm, g_gm

                def gru_mm(psum, wi, wh, bias_row, bcol0, bn):
                    mms = []
                    if wi is not None:
                        mms += [(pf[:, ki, :], wi[ki][:, bcol0 : bcol0 + bn])
                                for ki in range(2)]
                    if wh is not None:
                        mms += [(gf[:, ki, :], wh[ki][:, bcol0 : bcol0 + bn])
                                for ki in range(2)]
                    for i, (lhsT, rhs) in enumerate(mms):
                        nc.tensor.matmul(
                            psum[:], lhsT, rhs, start=(i == 0), stop=False,
                            skip_group_check=True,
                        )
                    nc.tensor.matmul(
                        psum[:], ones_row[:], bias_row, start=False, stop=True,
                        skip_group_check=True,
                    )

                rp = ps_h1.tile([GBLK, H], fp32, tag="psh1")
                gru_mm(rp, wih, whh, brz[:, 0:H], 0, H)
                zp = ps_h1.tile([GBLK, H], fp32, tag="psh1")
                gru_mm(zp, wih, whh, brz[:, H : 2 * H], H, H)
                inp_ = ps_tr.tile([GBLK, H], fp32, tag="pstr")
                gru_mm(inp_, wih, None, bin_[:], 2 * H, H)
                hnp = ps_tr.tile([GBLK, H], fp32, tag="pstr")
                gru_mm(hnp, None, whh, bhn[:], 2 * H, H)

                r = smallsb.tile([GBLK, H], fp32, tag="gru_r")
                nc.scalar.activation(r[:], rp[:], mybir.ActivationFunctionType.Sigmoid)
                z = smallsb.tile([GBLK, H], fp32, tag="gru_z")
                nc.scalar.activation(z[:], zp[:], mybir.ActivationFunctionType.Sigmoid)
                t1 = smallsb.tile([GBLK, H], fp32, tag="gru_s1")
                nc.vector.tensor_mul(t1[:], r[:], hnp[:])
                t2 = smallsb.tile([GBLK, H], fp32, tag="gru_s2")
                nc.vector.tensor_add(t2[:], t1[:], inp_[:])
                n = smallsb.tile([GBLK, H], fp32, tag="gru_n")
                nc.scalar.activation(n[:], t2[:], mybir.ActivationFunctionType.Tanh)
                t3 = smallsb.tile([GBLK, H], fp32, tag="gru_s1")
                nc.vector.tensor_sub(t3[:], h_old[:], n[:])
                t4 = smallsb.tile([GBLK, H], fp32, tag="gru_s2")
                nc.vector.tensor_mul(t4[:], z[:], t3[:])
                t5 = smallsb.tile([GBLK, H], fp32, tag="gru_s3")
                nc.vector.tensor_add(t5[:], n[:], t4[:])
                g_gm = gsb.tile([GBLK, H], fp32, tag="gsb")
                nc.scalar.activation(g_gm[:], t5[:],
                                     mybir.ActivationFunctionType.Relu)
                if ts < NUM_TIMESTEPS - 1:
                    g_fm = fm_copy(g_gm[:], gsb, "gfm", bf16)

            nc.sync.dma_start(out_d[j * GBLK : (j + 1) * GBLK, :], g_gm[:])

    nc.compile()
    return nc


def _prep_inputs(x, batch, counts, n_cores, nblk, NT=None):
    """Host-side shard + pad + layout. Returns (per_core, NT)."""
    import ml_dtypes

    G = n_cores * nblk * GBLK
    batch = np.asarray(batch).astype(np.int64)
    x = np.asarray(x, dtype=np.float32)

    edges = np.searchsorted(batch, np.arange(0, G + 1, GBLK))
    blk_cnt = np.diff(edges)
    if NT is None:
        NT = int(np.ceil(blk_cnt.max() / 128))
        NT = ((NT + LCHUNK - 1) // LCHUNK) * LCHUNK
    NTP = NT * 128

    invc_all = (1.0 / np.maximum(counts, 1.0)).astype(np.float32)

    xb = x.astype(ml_dtypes.bfloat16)
    per_core = []
    for k in range(n_cores):
        xk = np.zeros((nblk * NTP, H), dtype=ml_dtypes.bfloat16)
        bcols = np.full((nblk, 128, NT), -1.0, dtype=np.float32)
        for j in range(nblk):
            bi = k * nblk + j
            lo, hi = edges[bi], edges[bi + 1]
            cnt = hi - lo
            xk[j * NTP : j * NTP + cnt] = xb[lo:hi]
            blp = np.full(NTP, -1.0, dtype=np.float32)
            blp[:cnt] = (batch[lo:hi] - (bi * GBLK)).astype(np.float32)
            bcols[j] = blp.reshape(NT, 128).T
        invc = invc_all[k * nblk * GBLK : (k + 1) * nblk * GBLK].reshape(
            nblk, GBLK, 1
        )
        per_core.append({"xk": xk, "bcols": bcols, "invc": np.ascontiguousarray(invc)})
    return per_core, NT


def _const_inputs(gate_w1, gate_b1, gate_w2, gate_b2, gru_w_ih, gru_w_hh,
                  gru_b_ih, gru_b_hh):
    import ml_dtypes

    f = np.float32
    bf = ml_dtypes.bfloat16
    c = {}
    c["w1t"] = np.ascontiguousarray(
        np.asarray(gate_w1, f).T.reshape(2, 128, H)).astype(bf)
    c["b1row"] = np.asarray(gate_b1, f).reshape(1, H)
    c["w2bc"] = np.tile(np.asarray(gate_w2, f).reshape(1, H), (128, 1)).astype(bf)
    c["b2col"] = np.full((128, 1), np.asarray(gate_b2, f).reshape(()), dtype=f)
    c["wih_t"] = np.ascontiguousarray(
        np.asarray(gru_w_ih, f).T).reshape(2, 128, 3 * H).astype(bf)
    c["whh_t"] = np.ascontiguousarray(
        np.asarray(gru_w_hh, f).T).reshape(2, 128, 3 * H).astype(bf)
    bih = np.asarray(gru_b_ih, f)
    bhh = np.asarray(gru_b_hh, f)
    c["bsum_rz"] = (bih[: 2 * H] + bhh[: 2 * H]).reshape(1, 2 * H)
    c["bihn"] = bih[2 * H :].reshape(1, H)
    c["bhhn"] = bhh[2 * H :].reshape(1, H)
    c["iota_row"] = np.tile(np.arange(128, dtype=f), (128, 1))
    c["eye128"] = np.eye(128, dtype=f)
    c["eye128b"] = np.eye(128, dtype=f).astype(bf)
    return c


_CACHE = {}


def run(x, gate_w1, gate_b1, gate_w2, gate_b2, gru_w_ih, gru_w_hh, gru_b_ih,
        gru_b_hh, batch, num_graphs, n_cores=8, nblk=NBLK, trace=False,
        use_sim=False):
    from concourse.bass_utils import run_bass_kernel_spmd

    batch = np.asarray(batch).astype(np.int64)
    G = n_cores * nblk * GBLK
    counts = np.bincount(batch, minlength=G).astype(np.float32)
    per_core, NT = _prep_inputs(x, batch, counts, n_cores, nblk)
    consts = _const_inputs(gate_w1, gate_b1, gate_w2, gate_b2, gru_w_ih,
                           gru_w_hh, gru_b_ih, gru_b_hh)
    in_maps = [{**consts, **pc} for pc in per_core]

    key = (NT, nblk, n_cores)
    if key not in _CACHE:
        _CACHE[key] = _build_program(NT, nblk=nblk)
    nc = _CACHE[key]

    if use_sim:
        from concourse.bass_interp import CoreSim

        outs = []
        for k in range(n_cores):
            sim = CoreSim(nc)
            for name, arr in in_maps[k].items():
                sim.tensor(name)[:] = arr
            sim.simulate()
            outs.append(np.array(sim.tensor("out")))
        return np.concatenate(outs, axis=0), None

    res = run_bass_kernel_spmd(nc, in_maps, core_ids=list(range(n_cores)),
                               trace=trace)
    out = np.concatenate([res.results[k]["out"] for k in range(n_cores)], axis=0)
    return out, res


def kernel(**inputs):
    out, _ = run(**inputs)
    return out


# revision 14
# speedup vs baseline: 3.3940x; 1.6436x over previous
"""Trainium2 Bass kernel for AttentiveGraphPooling (gnn_message_passing).

Strategy: shard the 4096 graphs across 8 cores (512 graphs each). batch is
sorted, so each core owns a contiguous node range covering whole graphs ->
pooling / gather / GRU are all core-local, no collectives needed.

Per core, graphs go in 4 blocks of 128. Each block's node features (bf16,
node-major) are DMA'd once and stay resident in SBUF for all three passes
(init mean-pool + 2 timesteps); the host also supplies a pre-transposed
feature-major copy that is streamed per timestep for the W1 matmuls. A
per-tile one-hot E (node x local-graph, iota/is_equal on the vector engine)
turns the segment mean-pool into a TensorEngine matmul; E^T (for the
graph-context gather) is formed on the PE during phase A and kept resident.

The gate MLP uses distributivity to stay node-major with no per-node add:
  h1[n] = relu(W1 @ (x_n + g_b(n)) + b1) = relu(x_n @ W1^T + GW1[b(n)])
with GW1 = G @ W1^T + b1 built once per block/timestep. The gate scalar
sigmoid(h1 . w2 + b2) is computed batched over 4 node tiles (one relu /
mult / reduce / sigmoid per 4 tiles). The gate never multiplies x: it is
fused into the one-hot instead (Eg = is_equal(iota,b) * gate), so the
weighted pooling is Eg.T @ x. The GRU runs per graph-block with biases
folded in via K=1 matmuls.
"""

import os
import sys

import numpy as np

sys.path.insert(0, "/opt/trn_rl_repo")

H = 256
NBLK = 4  # graph blocks per core
GBLK = 128  # graphs per block
NUM_TIMESTEPS = 2
LCHUNK = 16  # node tiles per resident-load DMA
GB = 4  # gate batch (node tiles per batched gate pipeline)


def _build_program(NT, nblk=NBLK):
    """Build the single-core SPMD Bass program. NT = node tiles per block."""
    from contextlib import ExitStack

    import concourse.bass as bass
    import concourse.tile as tile
    from concourse import bacc, mybir

    fp32 = mybir.dt.float32
    bf16 = mybir.dt.bfloat16

    NTP = NT * 128  # padded nodes per block

    nc = bacc.Bacc("TRN2", target_bir_lowering=False, debug=False)

    # ---- DRAM parameters (per-core inputs) ----
    x_d = nc.dram_tensor("xk", [nblk * NTP, H], bf16, kind="ExternalInput")
    xt_d = nc.dram_tensor("xkT", [nblk, 2, 128, NTP], bf16, kind="ExternalInput")
    bcols_d = nc.dram_tensor("bcols", [nblk, 128, NT], fp32, kind="ExternalInput")
    invc_d = nc.dram_tensor("invc", [nblk, GBLK, 1], fp32, kind="ExternalInput")
    w1t_d = nc.dram_tensor("w1t", [2, 128, H], bf16, kind="ExternalInput")
    b1r_d = nc.dram_tensor("b1row", [1, H], fp32, kind="ExternalInput")
    w2bc_d = nc.dram_tensor("w2bc", [128, GB, H], bf16, kind="ExternalInput")
    b2c_d = nc.dram_tensor("b2col", [128, 1], fp32, kind="ExternalInput")
    wih_d = nc.dram_tensor("wih_t", [2, 128, 3 * H], bf16, kind="ExternalInput")
    whh_d = nc.dram_tensor("whh_t", [2, 128, 3 * H], bf16, kind="ExternalInput")
    brz_d = nc.dram_tensor("bsum_rz", [1, 2 * H], fp32, kind="ExternalInput")
    bin_d = nc.dram_tensor("bihn", [1, H], fp32, kind="ExternalInput")
    bhn_d = nc.dram_tensor("bhhn", [1, H], fp32, kind="ExternalInput")
    iota_d = nc.dram_tensor("iota_row", [128, 128], fp32, kind="ExternalInput")
    eye_d = nc.dram_tensor("eye128", [128, 128], fp32, kind="ExternalInput")
    eyeb_d = nc.dram_tensor("eye128b", [128, 128], bf16, kind="ExternalInput")
    out_d = nc.dram_tensor("out", [nblk * GBLK, H], fp32, kind="ExternalOutput")

    with tile.TileContext(nc) as tc, ExitStack() as ctx:
        ep = ctx.enter_context  # shorthand

        const = ep(tc.tile_pool(name="const", bufs=1))
        xres = ep(tc.tile_pool(name="xres", bufs=1))
        etres = ep(tc.tile_pool(name="etres", bufs=1))
        xtstr = ep(tc.tile_pool(name="xtstr", bufs=3))
        bpool = ep(tc.tile_pool(name="bcols", bufs=2))
        epool = ep(tc.tile_pool(name="eoh", bufs=6))
        trsh = ep(tc.tile_pool(name="trsh", bufs=2))
        gtsb = ep(tc.tile_pool(name="gtsb", bufs=3))
        gsb = ep(tc.tile_pool(name="gsb", bufs=3))
        smallsb = ep(tc.tile_pool(name="smallsb", bufs=2))

        ps_pool = ep(tc.tile_pool(name="pspool", bufs=2, space="PSUM"))
        ps_h1 = ep(tc.tile_pool(name="psh1", bufs=2, space="PSUM"))
        ps_et = ep(tc.tile_pool(name="pset", bufs=2, space="PSUM"))

        # ---- load constants ----
        def cload(shape, src, tag, dt=fp32):
            t = const.tile(shape, dt, tag=tag)
            nc.sync.dma_start(t[:], src)
            return t

        iota_row = cload([128, 128], iota_d[:], "c_iota")
        eye = cload([128, 128], eye_d[:], "c_eye")
        eyeb = cload([128, 128], eyeb_d[:], "c_eyeb", bf16)
        w1t = [cload([128, H], w1t_d[k], f"c_w1t{k}", bf16) for k in range(2)]
        b1row = cload([1, H], b1r_d[:], "c_b1r")
        w2bc = cload([128, GB, H], w2bc_d[:], "c_w2bc", bf16)
        b2col = cload([128, 1], b2c_d[:], "c_b2c")
        wih = [cload([128, 3 * H], wih_d[k], f"c_wih{k}", bf16) for k in range(2)]
        whh = [cload([128, 3 * H], whh_d[k], f"c_whh{k}", bf16) for k in range(2)]
        brz = cload([1, 2 * H], brz_d[:], "c_brz")
        bin_ = cload([1, H], bin_d[:], "c_bin")
        bhn = cload([1, H], bhn_d[:], "c_bhn")
        invc = [cload([GBLK, 1], invc_d[j], f"c_invc{j}") for j in range(nblk)]
        ones_row = const.tile([1, 128], fp32)
        nc.vector.memset(ones_row[:], 1.0)

        def fm_copy(g_ap, pool, tag, dt):
            """(128,256) graph-major -> feature-major (128,2,128) via PE."""
            gf = pool.tile([128, 2, GBLK], dt, tag=tag)
            for ki in range(2):
                tp = ps_et.tile([128, 128], fp32, tag="pset")
                nc.tensor.matmul(tp[:], g_ap[:, ki * 128 : (ki + 1) * 128], eye[:],
                                 is_transpose=True, start=True, stop=True)
                nc.scalar.copy(gf[:, ki, :], tp[:])
            return gf

        for j in range(nblk):
            # ---- resident x (node-major) for this block ----
            xj = xres.tile([128, NT, H], bf16, tag="xres")
            for c0 in range(0, NT, LCHUNK):
                base = j * NTP + c0 * 128
                src = x_d[base : base + LCHUNK * 128, :].rearrange(
                    "(c p) h -> p c h", p=128
                )
                nc.sync.dma_start(xj[:, c0 : c0 + LCHUNK, :], src)
            bt = bpool.tile([128, NT], fp32, tag="bcols")
            nc.sync.dma_start(bt[:], bcols_d[j])

            etj = etres.tile([128, NT, 128], bf16, tag="etres")

            # ---- phase A: initial mean pool + resident E^T build ----
            pooled = ps_pool.tile([GBLK, H], fp32, tag="pspool")
            for t in range(NT):
                e = epool.tile([128, 128], bf16, tag="eoh")
                nc.vector.tensor_scalar(
                    e[:], iota_row[:], bt[:, t : t + 1], None,
                    op0=mybir.AluOpType.is_equal,
                )
                nc.tensor.matmul(
                    pooled[:], e[:], xj[:, t, :], start=(t == 0), stop=(t == NT - 1),
                    skip_group_check=True,
                )
                tp = ps_et.tile([128, 128], bf16, tag="pset")
                nc.tensor.matmul(tp[:], e[:], eyeb[:], is_transpose=True,
                                 start=True, stop=True)
                nc.scalar.copy(etj[:, t, :], tp[:])
            g_gm = gsb.tile([GBLK, H], fp32, tag="gsb")
            nc.vector.tensor_scalar(
                g_gm[:], pooled[:], invc[j][:], None, op0=mybir.AluOpType.mult
            )
            g_fm = fm_copy(g_gm[:], gsb, "gfm", bf16)

            # ---- timesteps ----
            for ts in range(NUM_TIMESTEPS):
                # GW1 = G @ W1^T + b1  (graph-level, bf16)
                gw1p = ps_et.tile([GBLK, H], fp32, tag="pset")
                for ki in range(2):
                    nc.tensor.matmul(gw1p[:], g_fm[:, ki, :], w1t[ki][:],
                                     start=(ki == 0), stop=False,
                                     skip_group_check=True)
                nc.tensor.matmul(gw1p[:], ones_row[:], b1row[:],
                                 start=False, stop=True, skip_group_check=True)
                gw1 = gsb.tile([GBLK, H], bf16, tag="gw1")
                nc.scalar.copy(gw1[:], gw1p[:])

                pooled = ps_pool.tile([GBLK, H], fp32, tag="pspool")
                for t0 in range(0, NT, GB):
                    # stream x^T chunks (feature-major)
                    xts = xtstr.tile([128, 2, GB, 128], bf16, tag="xtstr")
                    for ki in range(2):
                        src = xt_d[j, ki, :, t0 * 128 : (t0 + GB) * 128].rearrange(
                            "p (c n) -> p c n", n=128
                        )
                        nc.sync.dma_start(xts[:, ki, :, :], src)
                    # h1 for GB tiles into one 2-bank PSUM tile
                    h1p = ps_h1.tile([128, GB, H], fp32, tag="psh1")
                    for c in range(GB):
                        t = t0 + c
                        for ki in range(2):
                            nc.tensor.matmul(
                                h1p[:, c, :], xts[:, ki, c, :], w1t[ki][:],
                                start=(ki == 0), stop=False,
                            )
                        nc.tensor.matmul(h1p[:, c, :], etj[:, t, :], gw1[:],
                                         start=False, stop=True)
                    # batched gate: relu -> .w2 -> rowsum -> sigmoid
                    # (relu split so each PSUM read stays within one bank)
                    trash = trsh.tile([128, GB, H], bf16, tag="trsh")
                    hb = GB // 2
                    nc.scalar.activation(trash[:, :hb, :], h1p[:, :hb, :],
                                         mybir.ActivationFunctionType.Relu)
                    nc.scalar.activation(trash[:, hb:, :], h1p[:, hb:, :],
                                         mybir.ActivationFunctionType.Relu)
                    prod = trsh.tile([128, GB, H], bf16, tag="prod")
                    nc.vector.tensor_mul(prod[:], trash[:], w2bc[:])
                    gpre = gtsb.tile([128, GB, 1], fp32, tag="gpre")
                    nc.vector.reduce_sum(gpre[:], prod[:], mybir.AxisListType.X)
                    gt = gtsb.tile([128, GB], fp32, tag="gtsb")
                    nc.scalar.activation(
                        gt[:], gpre[:, :, 0], mybir.ActivationFunctionType.Sigmoid,
                        bias=b2col[:],
                    )
                    # Eg = onehot * gate ; pooled += Eg.T @ x
                    for c in range(GB):
                        t = t0 + c
                        eg = epool.tile([128, 128], bf16, tag="eoh")
                        nc.vector.tensor_scalar(
                            eg[:], iota_row[:], bt[:, t : t + 1], gt[:, c : c + 1],
                            op0=mybir.AluOpType.is_equal, op1=mybir.AluOpType.mult,
                        )
                        nc.tensor.matmul(
                            pooled[:], eg[:], xj[:, t, :],
                            start=(t == 0), stop=(t == NT - 1),
                            skip_group_check=True,
                        )
                ps = gsb.tile([GBLK, H], fp32, tag="poolsb")
                nc.vector.tensor_scalar(
                    ps[:], pooled[:], invc[j][:], None, op0=mybir.AluOpType.mult
                )
                pf = fm_copy(ps[:], gsb, "poolfm", bf16)

                # ---- GRU cell (graph-major) ----
                gf, h_old = g_fm, g_gm

                def gru_mm(psum, wi, wh, bias_row, bcol0, bn):
                    mms = []
                    if wi is not None:
                        mms += [(pf[:, ki, :], wi[ki][:, bcol0 : bcol0 + bn])
                                for ki in range(2)]
                    if wh is not None:
                        mms += [(gf[:, ki, :], wh[ki][:, bcol0 : bcol0 + bn])
                                for ki in range(2)]
                    for i, (lhsT, rhs) in enumerate(mms):
                        nc.tensor.matmul(
                            psum[:], lhsT, rhs, start=(i == 0), stop=False,
                            skip_group_check=True,
                        )
                    nc.tensor.matmul(
                        psum[:], ones_row[:], bias_row, start=False, stop=True,
                        skip_group_check=True,
                    )

                rp = ps_h1.tile([GBLK, H], fp32, tag="psh1")
                gru_mm(rp, wih, whh, brz[:, 0:H], 0, H)
                zp = ps_h1.tile([GBLK, H], fp32, tag="psh1")
                gru_mm(zp, wih, whh, brz[:, H : 2 * H], H, H)
                inp_ = ps_et.tile([GBLK, H], fp32, tag="pset")
                gru_mm(inp_, wih, None, bin_[:], 2 * H, H)
                hnp = ps_et.tile([GBLK, H], fp32, tag="pset")
                gru_mm(hnp, None, whh, bhn[:], 2 * H, H)

                r = smallsb.tile([GBLK, H], fp32, tag="gru_r")
                nc.scalar.activation(r[:], rp[:], mybir.ActivationFunctionType.Sigmoid)
                z = smallsb.tile([GBLK, H], fp32, tag="gru_z")
                nc.scalar.activation(z[:], zp[:], mybir.ActivationFunctionType.Sigmoid)
                t1 = smallsb.tile([GBLK, H], fp32, tag="gru_s1")
                nc.vector.tensor_mul(t1[:], r[:], hnp[:])
                t2 = smallsb.tile([GBLK, H], fp32, tag="gru_s2")
                nc.vector.tensor_add(t2[:], t1[:], inp_[:])
                n = smallsb.tile([GBLK, H], fp32, tag="gru_n")
                nc.scalar.activation(n[:], t2[:], mybir.ActivationFunctionType.Tanh)
                t3 = smallsb.tile([GBLK, H], fp32, tag="gru_s1")
                nc.vector.tensor_sub(t3[:], h_old[:], n[:])
                t4 = smallsb.tile([GBLK, H], fp32, tag="gru_s2")
                nc.vector.tensor_mul(t4[:], z[:], t3[:])
                t5 = smallsb.tile([GBLK, H], fp32, tag="gru_s3")
                nc.vector.tensor_add(t5[:], n[:], t4[:])
                g_gm = gsb.tile([GBLK, H], fp32, tag="gsb")
                nc.scalar.activation(g_gm[:], t5[:],
                                     mybir.ActivationFunctionType.Relu)
                if ts < NUM_TIMESTEPS - 1:
                    g_fm = fm_copy(g_gm[:], gsb, "gfm", bf16)

            nc.sync.dma_start(out_d[j * GBLK : (j + 1) * GBLK, :], g_gm[:])

    nc.compile()
    return nc


def _prep_inputs(x, batch, counts, n_cores, nblk, NT=None):
    """Host-side shard + pad + layout. Returns (per_core, NT)."""
    import ml_dtypes

    G = n_cores * nblk * GBLK
    batch = np.asarray(batch).astype(np.int64)
    x = np.asarray(x, dtype=np.float32)

    edges = np.searchsorted(batch, np.arange(0, G + 1, GBLK))
    blk_cnt = np.diff(edges)
    if NT is None:
        NT = int(np.ceil(blk_cnt.max() / 128))
        NT = ((NT + LCHUNK - 1) // LCHUNK) * LCHUNK
    NTP = NT * 128

    invc_all = (1.0 / np.maximum(counts, 1.0)).astype(np.float32)

    xb = x.astype(ml_dtypes.bfloat16)
    per_core = []
    for k in range(n_cores):
        xk = np.zeros((nblk * NTP, H), dtype=ml_dtypes.bfloat16)
        bcols = np.full((nblk, 128, NT), -1.0, dtype=np.float32)
        for j in range(nblk):
            bi = k * nblk + j
            lo, hi = edges[bi], edges[bi + 1]
            cnt = hi - lo
            xk[j * NTP : j * NTP + cnt] = xb[lo:hi]
            blp = np.full(NTP, -1.0, dtype=np.float32)
            blp[:cnt] = (batch[lo:hi] - (bi * GBLK)).astype(np.float32)
            bcols[j] = blp.reshape(NT, 128).T
        xkT = np.ascontiguousarray(
            xk.reshape(nblk, NTP, 2, 128).transpose(0, 2, 3, 1)
        )
        invc = invc_all[k * nblk * GBLK : (k + 1) * nblk * GBLK].reshape(
            nblk, GBLK, 1
        )
        per_core.append({"xk": xk, "xkT": xkT, "bcols": bcols,
                         "invc": np.ascontiguousarray(invc)})
    return per_core, NT


def _const_inputs(gate_w1, gate_b1, gate_w2, gate_b2, gru_w_ih, gru_w_hh,
                  gru_b_ih, gru_b_hh):
    import ml_dtypes

    f = np.float32
    bf = ml_dtypes.bfloat16
    c = {}
    c["w1t"] = np.ascontiguousarray(
        np.asarray(gate_w1, f).T.reshape(2, 128, H)).astype(bf)
    c["b1row"] = np.asarray(gate_b1, f).reshape(1, H)
    c["w2bc"] = np.tile(np.asarray(gate_w2, f).reshape(1, 1, H),
                        (128, GB, 1)).astype(bf)
    c["b2col"] = np.full((128, 1), np.asarray(gate_b2, f).reshape(()), dtype=f)
    c["wih_t"] = np.ascontiguousarray(
        np.asarray(gru_w_ih, f).T).reshape(2, 128, 3 * H).astype(bf)
    c["whh_t"] = np.ascontiguousarray(
        np.asarray(gru_w_hh, f).T).reshape(2, 128, 3 * H).astype(bf)
    bih = np.asarray(gru_b_ih, f)
    bhh = np.asarray(gru_b_hh, f)
    c["bsum_rz"] = (bih[: 2 * H] + bhh[: 2 * H]).reshape(1, 2 * H)
    c["bihn"] = bih[2 * H :].reshape(1, H)
    c["bhhn"] = bhh[2 * H :].reshape(1, H)
    c["iota_row"] = np.tile(np.arange(128, dtype=f), (128, 1))
    c["eye128"] = np.eye(128, dtype=f)
    c["eye128b"] = np.eye(128, dtype=f).astype(bf)
    return c


_CACHE = {}


def run(x, gate_w1, gate_b1, gate_w2, gate_b2, gru_w_ih, gru_w_hh, gru_b_ih,
        gru_b_hh, batch, num_graphs, n_cores=8, nblk=NBLK, trace=False,
        use_sim=False):
    from concourse.bass_utils import run_bass_kernel_spmd

    batch = np.asarray(batch).astype(np.int64)
    G = n_cores * nblk * GBLK
    counts = np.bincount(batch, minlength=G).astype(np.float32)
    per_core, NT = _prep_inputs(x, batch, counts, n_cores, nblk)
    consts = _const_inputs(gate_w1, gate_b1, gate_w2, gate_b2, gru_w_ih,
                           gru_w_hh, gru_b_ih, gru_b_hh)
    in_maps = [{**consts, **pc} for pc in per_core]

    key = (NT, nblk, n_cores)
    if key not in _CACHE:
        _CACHE[key] = _build_program(NT, nblk=nblk)
    nc = _CACHE[key]

    if use_sim:
        from concourse.bass_interp import CoreSim

        outs = []
        for k in range(n_cores):
            sim = CoreSim(nc)
            for name, arr in in_maps[k].items():
                sim.tensor(name)[:] = arr
            sim.simulate()
            outs.append(np.array(sim.tensor("out")))
        return np.concatenate(outs, axis=0), None

    res = run_bass_kernel_spmd(nc, in_maps, core_ids=list(range(n_cores)),
                               trace=trace)
    out = np.concatenate([res.results[k]["out"] for k in range(n_cores)], axis=0)
    return out, res


def kernel(**inputs):
    out, _ = run(**inputs)
    return out
